# revision 28
# baseline (speedup 1.0000x reference)
"""Trainium2 Bass kernel for BERTSpanNER boundary scores (v4).

out[b,i,j,l] = min(cum[j+1,l]-cum[i,l], -EPS, begin[i,l], end[j,l]) for j>=i,
else -1e9, where cum/begin/end derive from log_softmax(x @ W + b) per label's
I,B,L,U tag group.

Sharding: 8 cores = 4 batches x 2 label-halves (8 labels each), SPMD.

v4 design (trace-driven rework of v2):
- PSUM as two 4-bank supertiles SA/SB [128, 2048] f32; sweep matmuls put
  label l's [128, mw] outer-subtract at bank offset (l%4)*512.
- A-half evacuation fused with min-G: ONE DVE tensor_tensor over the strided
  4-bank AP with a stride-0-free broadcast of g64. B-half: ONE scalar
  ACTIVATE Copy over its 4-bank AP + separate near-only DVE min-G (mid cols
  need no G-min: far-field hnh <= -120 << G).
- No negated C staging: the K=6 outer-subtract uses constant -1.0 rows in
  the rhs (A8) against +hml rows in the lhsT (N8), so both A8 and N8 stage
  from the same hml_d and the 3 scalar negate ops disappear.
- Prologue chunked in two 512-col halves end-to-end (logits/exp/sel/ln/ps24/
  scan) so the cumsum starts ~11us instead of ~25us; both act tables (Exp,
  Ln) are warmed at t=0; xk loads use 4 wide triggers.
- PE transposes live in SB cols [1920:2048], which no sweep matmul touches
  (l%4==3 writes [1536:1920]) -> zero PSUM WAR stalls; their ncs64/g64
  evacuations are 2+2 strided multi-tile ops, split 0-3 / 4-15 so tile-0
  sweep work is not gated on the full set.
- Broadcast-dependent ops (min-E2, deep fp16 adds, output DMA) lag one tile
  behind the evacuations so no engine queue ever stalls on the e2/a16
  broadcasts; deep splits 6 labels on Scalar (ACT Identity+bias), 2 on DVE.
- gpsimd does no elementwise work (its sw ops are slow AND stall concurrent
  DVE ops); it only memsets the +-1 constant rows and issues DMAs.
- Output: per-row-tile DRAM tensors out0..out7 [128, 8*(S-128t)] bf16 ->
  contiguous per-partition DMA runs (128 descriptors/tile vs 1024). Host
  reassembles [i, l, j] -> [i, j, l] and fills the j<i triangle with -1e9.
"""
import os
import sys

for _p in ("/opt/trn_rl_repo", "/root/.axon_site/_ro/trn_rl_repo"):
    if os.path.isdir(_p) and _p not in sys.path:
        sys.path.insert(0, _p)

import numpy as np
import concourse.bacc as bacc
import concourse.mybir as mybir
from concourse.bass import _add_dep_helper
from concourse.tile import TileContext
from concourse.bass_utils import run_bass_kernel_spmd
from concourse.alu_op_type import AluOpType

F32 = mybir.dt.float32
BF16 = mybir.dt.bfloat16
FP16 = mybir.dt.float16
AF = mybir.ActivationFunctionType

B, S, H, NL = 4, 1024, 400, 16
NT = 1 + 4 * NL          # 65
EPS = 1e-8
NEG = -1e9
P = 128
NST = S // P             # 8 row tiles
LC = NL // 2             # 8 labels per core
KT = [128, 128, 128, 17]  # k-tiling of H+1=401
NEARL = 192              # cols [i0, i0+NEARL) get the full 3-way min
DEEPL = 384              # cols [i0+DEEPL, S) read fp16 A (spans >= 257)
AD = S - DEEPL           # 640 deep cols max

_CACHED_NC = None


def _build():
    nc = bacc.Bacc()
    xk = nc.declare_dram_parameter("xk", [P, 4 * S], BF16, isOutput=False)
    Wk = nc.declare_dram_parameter("Wk", [P, 4 * NT], BF16, isOutput=False)
    selc = nc.declare_dram_parameter("selc", [P, 32], BF16, isOutput=False)
    sel2c = nc.declare_dram_parameter("sel2c", [P, 96], FP16, isOutput=False)
    eye = nc.declare_dram_parameter("eye", [P, P], F32, isOutput=False)
    # one output tensor; tile t's [P, LC*(S-128t)] block starts at col off[t]
    OUT_OFF = [0]
    for t in range(NST):
        OUT_OFF.append(OUT_OFF[-1] + LC * (S - P * t))
    out_d = nc.declare_dram_parameter("out", [P, OUT_OFF[-1]], BF16,
                                      isOutput=True)

    e2_row_d = nc.dram_tensor("e2_row_d", [LC, S], BF16)
    a16_row_d = nc.dram_tensor("a16_row_d", [LC, AD], FP16)
    hml_d = nc.dram_tensor("hml_d", [3 * LC, S + 1], BF16)

    with TileContext(nc) as tc:
        with tc.tile_pool(name="const", bufs=1) as cpool, \
             tc.tile_pool(name="work", bufs=1) as wpool, \
             tc.tile_pool(name="oc", bufs=1) as opool, \
             tc.tile_pool(name="ps", bufs=1, space="PSUM") as pspool:

            # warm the Exp act table before data arrives (the ACT has ONE
            # table slot: warming Ln too just forces extra 1.3us reloads)
            dm = cpool.tile([1, 1], F32, tag="dm")
            nc.vector.memset(dm[:], 0.0)
            dmo = cpool.tile([1, 1], F32, tag="dmo")
            nc.scalar.activation(dmo[:], dm[:], AF.Exp)

            # ---------------- input loads ------------------------------------
            # c0 column-halves of every k-chunk first so the chunk-0 prologue
            # can start ~2us earlier
            xk_sb = cpool.tile([P, 4 * S], BF16, tag="xk_sb")
            ring3 = [nc.sync, nc.scalar, nc.gpsimd]
            for c in range(2):
                for ki in range(4):
                    rows = KT[ki]
                    sl = slice(ki * S + c * 512, ki * S + c * 512 + 512)
                    ring3[ki % 3].dma_start(out=xk_sb[0:rows, sl],
                                            in_=xk[0:rows, sl])
            wk_sb = cpool.tile([P, 4 * NT], BF16, tag="wk_sb")
            nc.sync.dma_start(out=wk_sb[:], in_=Wk[:])
            selc_sb = cpool.tile([P, 32], BF16, tag="selc_sb")
            nc.scalar.dma_start(out=selc_sb[:], in_=selc[:])
            eye_sb = cpool.tile([P, P], F32, tag="eye_sb")
            nc.gpsimd.dma_start(out=eye_sb[:], in_=eye[:])
            sel2c_sb = cpool.tile([P, 96], FP16, tag="sel2c_sb")
            nc.scalar.dma_start(out=sel2c_sb[:], in_=sel2c[:])

            # A8 (rhs: hml rows + const -1) / N8 (lhsT: const 1 + hml rows)
            A8 = [wpool.tile([P, S + 1], BF16, name="a8%d" % h, tag="a8%d" % h)
                  for h in range(3)]
            N8 = [wpool.tile([P, S + 1], BF16, name="n8%d" % h, tag="n8%d" % h)
                  for h in range(3)]
            for h in range(3):
                nc.gpsimd.memset(A8[h][:], -1.0)
                nc.gpsimd.memset(N8[h][:], 1.0)

            SA = pspool.tile([P, 2048], F32, tag="SA")
            SB = pspool.tile([P, 2048], F32, tag="SB")

            # ---------------- prologue, chunked by 512-col halves ------------
            expT = wpool.tile([NT, S], BF16, tag="expT")
            lnsb = wpool.tile([32, S], FP16, tag="lnsb")
            gsb = wpool.tile([LC, S], F32, tag="gsb")
            e2sb = wpool.tile([LC, S], BF16, tag="e2sb")
            asb = wpool.tile([LC, S + 1], F32, tag="asb")
            nc.vector.memset(asb[:, 0:1], 0.0)
            exp_ins = []

            def stage1(c):       # logits + exp + tag-group sums
                cs = slice(c * 512, (c + 1) * 512)
                for ki, kt in enumerate(KT):
                    nc.tensor.matmul(
                        SA[0:NT, cs],
                        wk_sb[0:kt, ki * NT:(ki + 1) * NT],
                        xk_sb[0:kt, ki * S + c * 512: ki * S + c * 512 + 512],
                        start=ki == 0, stop=ki == 3)
                ei = nc.scalar.activation(expT[:, cs], SA[0:NT, cs], AF.Exp)
                exp_ins.append(ei)
                nc.tensor.matmul(SA[0:32, 1024 + c * 512:1536 + c * 512],
                                 selc_sb[0:NT, :], expT[:, cs],
                                 start=True, stop=True)

            def stage2(c):       # ln + ps24 + cumsum chunks
                cs = slice(c * 512, (c + 1) * 512)
                li = nc.scalar.activation(
                    lnsb[:25, cs], SA[0:25, 1024 + c * 512:1536 + c * 512],
                    AF.Ln)
                _add_dep_helper(li.ins, exp_ins[-1].ins, True,
                                "one act-table switch: all exps before lns")
                nc.tensor.matmul(SB[0:96, cs], sel2c_sb[0:25, :],
                                 lnsb[:25, cs], start=True, stop=True)
                for q in (2 * c, 2 * c + 1):
                    qs = q * 256
                    nc.vector.tensor_tensor_scan(
                        asb[:, 1 + qs:1 + qs + 256],
                        SB[0:LC, qs:qs + 256],
                        expT[0:LC, 0:256],
                        0.0 if q == 0 else asb[:, qs:qs + 1],
                        AluOpType.add, AluOpType.bypass)

            stage1(0)
            stage1(1)
            stage2(0)
            stage2(1)
            for c in range(2):
                cs = slice(c * 512, (c + 1) * 512)
                nc.scalar.activation(e2sb[:, cs], SB[64:72, cs], AF.Copy)
                nc.scalar.activation(gsb[:, cs], SB[32:40, cs], AF.Copy)
            dma_w_e = nc.scalar.dma_start(out=e2_row_d[:], in_=e2sb[:])

            # fp16 copy of the deep A cols + its broadcast staging
            a16sb = wpool.tile([LC, AD], FP16, tag="a16sb")
            nc.vector.tensor_copy(a16sb[:], asb[:, DEEPL + 1:S + 1])
            dma_w_a16 = nc.gpsimd.dma_start(out=a16_row_d[:], in_=a16sb[:])

            # ------- triple-bf16 split of A + A8/N8 staging ------------------
            hml = wpool.tile([P, S + 1], BF16, tag="hml")
            r1 = wpool.tile([LC, S + 1], F32, tag="r1")
            r2 = wpool.tile([LC, S + 1], F32, tag="r2")
            mid0 = wpool.tile([LC, S + 1], BF16, tag="mid0")
            nc.vector.tensor_copy(hml[0:LC, :], asb[:])
            nc.vector.tensor_tensor(r1[:], asb[:], hml[0:LC, :],
                                    AluOpType.subtract)
            nc.vector.tensor_copy(mid0[:], r1[:])
            nc.vector.tensor_tensor(r2[:], r1[:], mid0[:],
                                    AluOpType.subtract)
            nc.vector.tensor_copy(hml[32:32 + LC, :], mid0[:])
            nc.vector.tensor_copy(hml[64:64 + LC, :], r2[:])

            w_hml = []
            for k in range(3):
                w_hml.append(ring3[k].dma_start(
                    out=hml_d[8 * k:8 * k + 8, :],
                    in_=hml[32 * k:32 * k + LC, :]))
            hml3 = hml_d[:].rearrange("(k l) j -> l k j", l=LC)
            stage_reads = []
            for l in range(LC):
                h, goff = l // 3, 32 * (l % 3)
                ra = ring3[l % 3].dma_start(out=A8[h][goff:goff + 3, :],
                                            in_=hml3[l, :, :])
                rn = ring3[(l + 1) % 3].dma_start(
                    out=N8[h][goff + 3:goff + 6, :], in_=hml3[l, :, :])
                stage_reads += [ra, rn]
                for w in w_hml:
                    _add_dep_helper(ra.ins, w.ins, True, "a8 stage RAW")
                    _add_dep_helper(rn.ins, w.ins, True, "n8 stage RAW")

            # packed broadcasts AFTER the staging reads on each ring queue:
            # the 3.3MB of broadcast traffic must not starve the ~100KB of
            # staging that gates the first sweep matmul; the lag-1 sweep
            # structure gives the broadcasts ~10us to land before first use.
            E2h = [wpool.tile([P, 4 * S], BF16, name="e2_b%d" % h,
                              tag="e2_b%d" % h) for h in range(2)]
            E2A3 = E2h[0][:].rearrange("p (l j) -> p l j", l=4)
            E2B3 = E2h[1][:].rearrange("p (l j) -> p l j", l=4)
            for h in range(2):
                re = ring3[h].dma_start(
                    out=E2h[h][:],
                    in_=e2_row_d[4 * h:4 * h + 4, :].rearrange(
                        "l j -> (l j)").partition_broadcast(P))
                _add_dep_helper(re.ins, dma_w_e.ins, True, "e2 bcast RAW")
                # Tile's scheduler orders triggers by dep readiness, not
                # emission order: pin the 1MB bcast behind the staging reads
                # so its data cannot starve the reads that gate the sweep
                for sr in stage_reads:
                    _add_dep_helper(re.ins, sr.ins, True, "bcast after stage")
            A16all = wpool.tile([P, LC * AD], FP16, tag="a16all")
            A163 = A16all[:].rearrange("p (l j) -> p l j", l=LC)
            ra16 = nc.gpsimd.dma_start(
                out=A16all[:],
                in_=a16_row_d[:].rearrange("l j -> (l j)").partition_broadcast(P))
            _add_dep_helper(ra16.ins, dma_w_a16.ins, True, "a16 RAW")
            for sr in stage_reads:
                _add_dep_helper(ra16.ins, sr.ins, True, "bcast after stage")

            # ------- C, G' per partition: transposes in SB[1920:2048] --------
            # (no sweep matmul touches those cols: l%4==3 writes [1536:1920])
            ncs64 = wpool.tile([P, NST * LC], F32, tag="ncs64")   # -C
            g64 = wpool.tile([P, NST * LC], BF16, tag="g64")      # min(G,-EPS)
            trc3 = SB[:, 1920:2048].rearrange(
                "p (t two l) -> p t two l", two=2, l=8)

            def transposes(t0, t1):
                for t in range(t0, t1):
                    nc.tensor.transpose(
                        SB[:P, 1920 + 16 * t:1920 + 16 * t + 8],
                        asb[:, t * P: t * P + P], eye_sb[0:LC, 0:LC])
                    nc.tensor.transpose(
                        SB[:P, 1928 + 16 * t:1928 + 16 * t + 8],
                        gsb[:, t * P: t * P + P], eye_sb[0:LC, 0:LC])
                nc.scalar.activation(
                    ncs64[:, t0 * 8:t1 * 8].rearrange(
                        "p (t l) -> p t l", l=8),
                    trc3[:, t0:t1, 0, :], AF.Copy, scale=-1.0)
                nc.vector.tensor_scalar(
                    g64[:, t0 * 8:t1 * 8].rearrange(
                        "p (t l) -> p t l", l=8),
                    trc3[:, t0:t1, 1, :], -EPS, None, AluOpType.min)

            transposes(0, 4)

            # ---------------- main sweep -------------------------------------
            SA3 = SA[:].rearrange("p (l c) -> p l c", l=4)
            SB3 = SB[:].rearrange("p (l c) -> p l c", l=4)
            ocs = []

            def mm(t, l, sup, mw):
                i0 = t * P
                h, goff = l // 3, 32 * (l % 3)
                nc.tensor.matmul(
                    sup[:, (l % 4) * 512:(l % 4) * 512 + mw],
                    N8[h][goff:goff + 6, i0:i0 + P],
                    A8[h][goff:goff + 6, 1 + i0:1 + i0 + mw],
                    start=True, stop=True)

            def lagged(t):
                i0 = t * P
                W = S - i0
                nw = min(NEARL, W)
                mw = min(DEEPL, W)
                dw = W - mw
                oc, oc3 = ocs[t]
                nc.vector.tensor_tensor(oc3[:, 0:4, 0:nw], oc3[:, 0:4, 0:nw],
                                        E2A3[:, :, i0:i0 + nw], AluOpType.min)
                nc.vector.tensor_tensor(oc3[:, 4:8, 0:nw], oc3[:, 4:8, 0:nw],
                                        E2B3[:, :, i0:i0 + nw], AluOpType.min)
                for l in (0, 1, 2, 4, 5, 6):
                    if dw:
                        nc.scalar.activation(
                            oc3[:, l, mw:W], A163[:, l, i0:i0 + dw],
                            AF.Identity, bias=ncs64[:, t * LC + l:
                                                    t * LC + l + 1])
                for l in (3, 7):
                    if dw:
                        nc.vector.tensor_scalar(
                            oc3[:, l, mw:W], A163[:, l, i0:i0 + dw],
                            ncs64[:, t * LC + l:t * LC + l + 1], None,
                            AluOpType.add)
                o0 = OUT_OFF[t]
                nc.sync.dma_start(out=out_d[:, o0:o0 + 4 * W],
                                  in_=oc[:, 0:4 * W])
                nc.gpsimd.dma_start(out=out_d[:, o0 + 4 * W:o0 + 8 * W],
                                    in_=oc[:, 4 * W:8 * W])

            for t in range(NST):
                i0 = t * P
                W = S - i0
                nw = min(NEARL, W)
                mw = min(DEEPL, W)
                oc = opool.tile([P, LC * W], BF16, name="oc%d" % t,
                                tag="oc%d" % t)
                oc3 = oc[:].rearrange("p (l j) -> p l j", j=W)
                ocs.append((oc, oc3))

                gA = g64[:, t * LC:t * LC + 4].rearrange(
                    "p l -> p l ()").broadcast_to([P, 4, mw])
                gBn = g64[:, t * LC + 4:t * LC + 8].rearrange(
                    "p l -> p l ()").broadcast_to([P, 4, nw])

                for l in range(4):
                    mm(t, l, SA, mw)
                # fused evacuation + min-G of the A half (mid cols too:
                # harmless, hnh << G there)
                nc.vector.tensor_tensor(oc3[:, 0:4, 0:mw],
                                        SA3[:, :, 0:mw], gA, AluOpType.min)
                for l in range(4, 8):
                    mm(t, l, SB, mw)
                nc.scalar.activation(oc3[:, 4:8, 0:mw], SB3[:, :, 0:mw],
                                     AF.Copy)
                nc.vector.tensor_tensor(oc3[:, 4:8, 0:nw], oc3[:, 4:8, 0:nw],
                                        gBn, AluOpType.min)
                if t == 0:
                    transposes(4, NST)
                if t > 0:
                    lagged(t - 1)
            lagged(NST - 1)

    nc.compile()
    return nc


def _bf16(a):
    u = np.ascontiguousarray(a, dtype=np.float32).view(np.uint32)
    r = ((u >> 16) & 1) + 0x7FFF
    return ((u + r) >> 16).astype(np.uint16)


def _unbf16(a):
    return (a.astype(np.uint32) << 16).view(np.float32)


def _host_inputs(x, W, b):
    """Per-core inputs. Core c: batch c//2, label half c%2."""
    x = np.asarray(x, dtype=np.float32)
    W = np.asarray(W, dtype=np.float32)
    b = np.asarray(b, dtype=np.float32)

    Wb = np.concatenate([W, b[None, :]], axis=0)          # (401, 65)
    wkp = np.zeros((4 * P, NT), np.float32)
    wkp[:H + 1] = Wb
    wk = _bf16(wkp.reshape(4, P, NT).transpose(1, 0, 2).reshape(P, 4 * NT))
    eye = np.eye(P, dtype=np.float32)
    sel2 = np.zeros((P, 96), np.float32)
    cols = np.concatenate([np.arange(8), 32 + np.arange(8), 64 + np.arange(8)])
    sel2[0, cols] = -1.0
    sel2[1 + np.arange(24), cols] = 1.0

    in_maps = []
    for c in range(8):
        bb, h = c // 2, c % 2
        xTb = np.concatenate([x[bb].T, np.ones((1, S), np.float32)], axis=0)
        xp = np.zeros((4 * P, S), np.float32)
        xp[:H + 1] = xTb
        xkc = _bf16(xp.reshape(4, P, S).transpose(1, 0, 2).reshape(P, 4 * S))
        sel = np.zeros((P, 32), np.float32)
        sel[:NT, 0] = 1.0
        for g in range(LC):
            lg = h * LC + g
            base = 1 + 4 * lg
            sel[base:base + 4, 1 + g] = 1.0          # I,B,L,U
            sel[[base + 1, base + 3], 9 + g] = 1.0   # B,U -> begin
            sel[[base + 2, base + 3], 17 + g] = 1.0  # L,U -> end
        in_maps.append({
            "xk": xkc, "Wk": wk, "selc": _bf16(sel),
            "sel2c": sel2.astype(np.float16), "eye": eye,
        })
    return in_maps


def kernel(x, mask, W, b, _collect=None):
    global _CACHED_NC
    if _CACHED_NC is None:
        _CACHED_NC = _build()
    nc = _CACHED_NC
    in_maps = _host_inputs(x, W, b)
    res = run_bass_kernel_spmd(nc, in_maps, list(range(8)))
    if _collect is not None:
        _collect.append(res)
    off = [0]
    for t in range(NST):
        off.append(off[-1] + LC * (S - P * t))
    outf = np.empty((B, S, S, NL), dtype=np.float32)
    for c in range(8):
        bb, h = c // 2, c % 2
        ofull = res.results[c]["out"]
        if ofull.dtype != np.uint16:
            ofull = ofull.view(np.uint16)
        for t in range(NST):
            i0 = t * P
            o = _unbf16(ofull[:, off[t]:off[t + 1]]).reshape(P, LC, S - i0)
            outf[bb, i0:i0 + P, i0:S, h * LC:(h + 1) * LC] = \
                o.transpose(0, 2, 1)
    # constant left/lower region (j < i) filled on host
    for i in range(1, S):
        outf[:, i, :i, :] = NEG
    return outf


# revision 29
# speedup vs baseline: 1.2278x; 1.2278x over previous
"""Trainium2 Bass kernel for BERTSpanNER boundary scores (v4).

out[b,i,j,l] = min(cum[j+1,l]-cum[i,l], -EPS, begin[i,l], end[j,l]) for j>=i,
else -1e9, where cum/begin/end derive from log_softmax(x @ W + b) per label's
I,B,L,U tag group.

Sharding: 8 cores = 4 batches x 2 label-halves (8 labels each), SPMD.

v4 design (trace-driven rework of v2):
- PSUM as two 4-bank supertiles SA/SB [128, 2048] f32; sweep matmuls put
  label l's [128, mw] outer-subtract at bank offset (l%4)*512.
- A-half evacuation fused with min-G: ONE DVE tensor_tensor over the strided
  4-bank AP with a stride-0-free broadcast of g64. B-half: ONE scalar
  ACTIVATE Copy over its 4-bank AP + separate near-only DVE min-G (mid cols
  need no G-min: far-field hnh <= -120 << G).
- No negated C staging: the K=6 outer-subtract uses constant -1.0 rows in
  the rhs (A8) against +hml rows in the lhsT (N8), so both A8 and N8 stage
  from the same hml_d and the 3 scalar negate ops disappear.
- Prologue chunked in two 512-col halves end-to-end (logits/exp/sel/ln/ps24/
  scan) so the cumsum starts ~11us instead of ~25us; both act tables (Exp,
  Ln) are warmed at t=0; xk loads use 4 wide triggers.
- PE transposes live in SB cols [1920:2048], which no sweep matmul touches
  (l%4==3 writes [1536:1920]) -> zero PSUM WAR stalls; their ncs64/g64
  evacuations are 2+2 strided multi-tile ops, split 0-3 / 4-15 so tile-0
  sweep work is not gated on the full set.
- Broadcast-dependent ops (min-E2, deep fp16 adds, output DMA) lag one tile
  behind the evacuations so no engine queue ever stalls on the e2/a16
  broadcasts; deep splits 6 labels on Scalar (ACT Identity+bias), 2 on DVE.
- gpsimd does no elementwise work (its sw ops are slow AND stall concurrent
  DVE ops); it only memsets the +-1 constant rows and issues DMAs.
- Output: per-row-tile DRAM tensors out0..out7 [128, 8*(S-128t)] bf16 ->
  contiguous per-partition DMA runs (128 descriptors/tile vs 1024). Host
  reassembles [i, l, j] -> [i, j, l] and fills the j<i triangle with -1e9.
"""
import os
import sys

for _p in ("/opt/trn_rl_repo", "/root/.axon_site/_ro/trn_rl_repo"):
    if os.path.isdir(_p) and _p not in sys.path:
        sys.path.insert(0, _p)

import numpy as np
import concourse.bacc as bacc
import concourse.mybir as mybir
from concourse.bass import _add_dep_helper
from concourse.tile import TileContext
from concourse.bass_utils import run_bass_kernel_spmd
from concourse.alu_op_type import AluOpType

F32 = mybir.dt.float32
BF16 = mybir.dt.bfloat16
FP16 = mybir.dt.float16
AF = mybir.ActivationFunctionType

B, S, H, NL = 4, 1024, 400, 16
NT = 1 + 4 * NL          # 65
EPS = 1e-8
NEG = -1e9
P = 128
NST = S // P             # 8 row tiles
LC = NL // 2             # 8 labels per core
KT = [128, 128, 128, 17]  # k-tiling of H+1=401
NEARL = 192              # cols [i0, i0+NEARL) get the full 3-way min
DEEPL = 384              # cols [i0+DEEPL, S) read fp16 A (spans >= 257)
AD = S - DEEPL           # 640 deep cols max

_CACHED_NC = None


def _build():
    nc = bacc.Bacc()
    xk = nc.declare_dram_parameter("xk", [P, 4 * S], BF16, isOutput=False)
    Wk = nc.declare_dram_parameter("Wk", [P, 4 * NT], BF16, isOutput=False)
    selc = nc.declare_dram_parameter("selc", [P, 32], BF16, isOutput=False)
    sel2c = nc.declare_dram_parameter("sel2c", [P, 96], FP16, isOutput=False)
    eye = nc.declare_dram_parameter("eye", [P, P], F32, isOutput=False)
    # one output tensor; tile t's [P, LC*(S-128t)] block starts at col off[t]
    OUT_OFF = [0]
    for t in range(NST):
        OUT_OFF.append(OUT_OFF[-1] + LC * (S - P * t))
    out_d = nc.declare_dram_parameter("out", [P, OUT_OFF[-1]], BF16,
                                      isOutput=True)

    e2_row_d = nc.dram_tensor("e2_row_d", [LC, S], BF16)
    a16_row_d = nc.dram_tensor("a16_row_d", [LC, AD], FP16)
    hml_d = nc.dram_tensor("hml_d", [3 * LC, S + 1], BF16)

    with TileContext(nc) as tc:
        with tc.tile_pool(name="const", bufs=1) as cpool, \
             tc.tile_pool(name="work", bufs=1) as wpool, \
             tc.tile_pool(name="oc", bufs=1) as opool, \
             tc.tile_pool(name="ps", bufs=1, space="PSUM") as pspool:

            # warm the Exp act table before data arrives (the ACT has ONE
            # table slot: warming Ln too just forces extra 1.3us reloads)
            dm = cpool.tile([1, 1], F32, tag="dm")
            nc.vector.memset(dm[:], 0.0)
            dmo = cpool.tile([1, 1], F32, tag="dmo")
            nc.scalar.activation(dmo[:], dm[:], AF.Exp)

            # ---------------- input loads ------------------------------------
            # c0 column-halves of every k-chunk first so the chunk-0 prologue
            # can start ~2us earlier
            xk_sb = cpool.tile([P, 4 * S], BF16, tag="xk_sb")
            ring3 = [nc.sync, nc.scalar, nc.gpsimd]
            for c in range(2):
                for ki in range(4):
                    rows = KT[ki]
                    sl = slice(ki * S + c * 512, ki * S + c * 512 + 512)
                    ring3[ki % 3].dma_start(out=xk_sb[0:rows, sl],
                                            in_=xk[0:rows, sl])
            wk_sb = cpool.tile([P, 4 * NT], BF16, tag="wk_sb")
            nc.sync.dma_start(out=wk_sb[:], in_=Wk[:])
            selc_sb = cpool.tile([P, 32], BF16, tag="selc_sb")
            nc.scalar.dma_start(out=selc_sb[:], in_=selc[:])
            eye_sb = cpool.tile([P, P], F32, tag="eye_sb")
            nc.gpsimd.dma_start(out=eye_sb[:], in_=eye[:])
            sel2c_sb = cpool.tile([P, 96], FP16, tag="sel2c_sb")
            nc.scalar.dma_start(out=sel2c_sb[:], in_=sel2c[:])

            # A8 (rhs: hml rows + const -1) / N8 (lhsT: const 1 + hml rows)
            A8 = [wpool.tile([P, S + 1], BF16, name="a8%d" % h, tag="a8%d" % h)
                  for h in range(3)]
            N8 = [wpool.tile([P, S + 1], BF16, name="n8%d" % h, tag="n8%d" % h)
                  for h in range(3)]
            for h in range(3):
                nc.gpsimd.memset(A8[h][:], -1.0)
                nc.gpsimd.memset(N8[h][:], 1.0)

            SA = pspool.tile([P, 2048], F32, tag="SA")
            SB = pspool.tile([P, 2048], F32, tag="SB")

            # ---------------- prologue, chunked by 512-col halves ------------
            expT = wpool.tile([NT, S], BF16, tag="expT")
            lnsb = wpool.tile([32, S], FP16, tag="lnsb")
            gsb = wpool.tile([LC, S], F32, tag="gsb")
            e2sb = wpool.tile([LC, S], BF16, tag="e2sb")
            asb = wpool.tile([LC, S + 1], F32, tag="asb")
            nc.vector.memset(asb[:, 0:1], 0.0)
            exp_ins = []

            def stage1(c):       # logits + exp + tag-group sums
                cs = slice(c * 512, (c + 1) * 512)
                for ki, kt in enumerate(KT):
                    nc.tensor.matmul(
                        SA[0:NT, cs],
                        wk_sb[0:kt, ki * NT:(ki + 1) * NT],
                        xk_sb[0:kt, ki * S + c * 512: ki * S + c * 512 + 512],
                        start=ki == 0, stop=ki == 3)
                ei = nc.scalar.activation(expT[:, cs], SA[0:NT, cs], AF.Exp)
                exp_ins.append(ei)
                nc.tensor.matmul(SA[0:32, 1024 + c * 512:1536 + c * 512],
                                 selc_sb[0:NT, :], expT[:, cs],
                                 start=True, stop=True)

            def stage2(c):       # ln + ps24 + cumsum chunks
                cs = slice(c * 512, (c + 1) * 512)
                li = nc.scalar.activation(
                    lnsb[:25, cs], SA[0:25, 1024 + c * 512:1536 + c * 512],
                    AF.Ln)
                _add_dep_helper(li.ins, exp_ins[-1].ins, True,
                                "one act-table switch: all exps before lns")
                nc.tensor.matmul(SB[0:96, cs], sel2c_sb[0:25, :],
                                 lnsb[:25, cs], start=True, stop=True)
                for q in (2 * c, 2 * c + 1):
                    qs = q * 256
                    nc.vector.tensor_tensor_scan(
                        asb[:, 1 + qs:1 + qs + 256],
                        SB[0:LC, qs:qs + 256],
                        expT[0:LC, 0:256],
                        0.0 if q == 0 else asb[:, qs:qs + 1],
                        AluOpType.add, AluOpType.bypass)

            stage1(0)
            stage1(1)
            stage2(0)
            stage2(1)
            for c in range(2):
                cs = slice(c * 512, (c + 1) * 512)
                nc.scalar.activation(e2sb[:, cs], SB[64:72, cs], AF.Copy)
                nc.scalar.activation(gsb[:, cs], SB[32:40, cs], AF.Copy)
            dma_w_e = nc.scalar.dma_start(out=e2_row_d[:], in_=e2sb[:])

            # fp16 copy of the deep A cols + its broadcast staging
            a16sb = wpool.tile([LC, AD], FP16, tag="a16sb")
            nc.vector.tensor_copy(a16sb[:], asb[:, DEEPL + 1:S + 1])
            dma_w_a16 = nc.gpsimd.dma_start(out=a16_row_d[:], in_=a16sb[:])

            # ------- triple-bf16 split of A + A8/N8 staging ------------------
            hml = wpool.tile([P, S + 1], BF16, tag="hml")
            r1 = wpool.tile([LC, S + 1], F32, tag="r1")
            r2 = wpool.tile([LC, S + 1], F32, tag="r2")
            mid0 = wpool.tile([LC, S + 1], BF16, tag="mid0")
            nc.vector.tensor_copy(hml[0:LC, :], asb[:])
            nc.vector.tensor_tensor(r1[:], asb[:], hml[0:LC, :],
                                    AluOpType.subtract)
            nc.vector.tensor_copy(mid0[:], r1[:])
            nc.vector.tensor_tensor(r2[:], r1[:], mid0[:],
                                    AluOpType.subtract)
            nc.vector.tensor_copy(hml[32:32 + LC, :], mid0[:])
            nc.vector.tensor_copy(hml[64:64 + LC, :], r2[:])

            w_hml = []
            for k in range(3):
                w_hml.append(ring3[k].dma_start(
                    out=hml_d[8 * k:8 * k + 8, :],
                    in_=hml[32 * k:32 * k + LC, :]))
            hml3 = hml_d[:].rearrange("(k l) j -> l k j", l=LC)
            for l in range(LC):
                h, goff = l // 3, 32 * (l % 3)
                ra = ring3[l % 3].dma_start(out=A8[h][goff:goff + 3, :],
                                            in_=hml3[l, :, :])
                rn = ring3[(l + 1) % 3].dma_start(
                    out=N8[h][goff + 3:goff + 6, :], in_=hml3[l, :, :])
                for w in w_hml:
                    _add_dep_helper(ra.ins, w.ins, True, "a8 stage RAW")
                    _add_dep_helper(rn.ins, w.ins, True, "n8 stage RAW")

            # packed broadcasts AFTER the staging reads on each ring queue:
            # the 3.3MB of broadcast traffic must not starve the ~100KB of
            # staging that gates the first sweep matmul; the lag-1 sweep
            # structure gives the broadcasts ~10us to land before first use.
            E2h = [wpool.tile([P, 4 * S], BF16, name="e2_b%d" % h,
                              tag="e2_b%d" % h) for h in range(2)]
            E2A3 = E2h[0][:].rearrange("p (l j) -> p l j", l=4)
            E2B3 = E2h[1][:].rearrange("p (l j) -> p l j", l=4)
            for h in range(2):
                re = ring3[h].dma_start(
                    out=E2h[h][:],
                    in_=e2_row_d[4 * h:4 * h + 4, :].rearrange(
                        "l j -> (l j)").partition_broadcast(P))
                _add_dep_helper(re.ins, dma_w_e.ins, True, "e2 bcast RAW")
            A16all = wpool.tile([P, LC * AD], FP16, tag="a16all")
            A163 = A16all[:].rearrange("p (l j) -> p l j", l=LC)
            ra16 = nc.gpsimd.dma_start(
                out=A16all[:],
                in_=a16_row_d[:].rearrange("l j -> (l j)").partition_broadcast(P))
            _add_dep_helper(ra16.ins, dma_w_a16.ins, True, "a16 RAW")

            # ------- C, G' per partition: transposes in SB[1920:2048] --------
            # (no sweep matmul touches those cols: l%4==3 writes [1536:1920])
            ncs64 = wpool.tile([P, NST * LC], F32, tag="ncs64")   # -C
            g64 = wpool.tile([P, NST * LC], BF16, tag="g64")      # min(G,-EPS)
            trc3 = SB[:, 1920:2048].rearrange(
                "p (t two l) -> p t two l", two=2, l=8)

            def transposes(t0, t1):
                for t in range(t0, t1):
                    nc.tensor.transpose(
                        SB[:P, 1920 + 16 * t:1920 + 16 * t + 8],
                        asb[:, t * P: t * P + P], eye_sb[0:LC, 0:LC])
                    nc.tensor.transpose(
                        SB[:P, 1928 + 16 * t:1928 + 16 * t + 8],
                        gsb[:, t * P: t * P + P], eye_sb[0:LC, 0:LC])
                nc.scalar.activation(
                    ncs64[:, t0 * 8:t1 * 8].rearrange(
                        "p (t l) -> p t l", l=8),
                    trc3[:, t0:t1, 0, :], AF.Copy, scale=-1.0)
                nc.vector.tensor_scalar(
                    g64[:, t0 * 8:t1 * 8].rearrange(
                        "p (t l) -> p t l", l=8),
                    trc3[:, t0:t1, 1, :], -EPS, None, AluOpType.min)

            transposes(0, 4)

            # ---------------- main sweep -------------------------------------
            SA3 = SA[:].rearrange("p (l c) -> p l c", l=4)
            SB3 = SB[:].rearrange("p (l c) -> p l c", l=4)
            ocs = []

            def mm(t, l, sup, mw):
                i0 = t * P
                h, goff = l // 3, 32 * (l % 3)
                nc.tensor.matmul(
                    sup[:, (l % 4) * 512:(l % 4) * 512 + mw],
                    N8[h][goff:goff + 6, i0:i0 + P],
                    A8[h][goff:goff + 6, 1 + i0:1 + i0 + mw],
                    start=True, stop=True)

            def lagged(t):
                i0 = t * P
                W = S - i0
                nw = min(NEARL, W)
                mw = min(DEEPL, W)
                dw = W - mw
                oc, oc3 = ocs[t]
                nc.vector.tensor_tensor(oc3[:, 0:4, 0:nw], oc3[:, 0:4, 0:nw],
                                        E2A3[:, :, i0:i0 + nw], AluOpType.min)
                nc.vector.tensor_tensor(oc3[:, 4:8, 0:nw], oc3[:, 4:8, 0:nw],
                                        E2B3[:, :, i0:i0 + nw], AluOpType.min)
                for l in (0, 1, 2, 4, 5, 6):
                    if dw:
                        nc.scalar.activation(
                            oc3[:, l, mw:W], A163[:, l, i0:i0 + dw],
                            AF.Identity, bias=ncs64[:, t * LC + l:
                                                    t * LC + l + 1])
                for l in (3, 7):
                    if dw:
                        nc.vector.tensor_scalar(
                            oc3[:, l, mw:W], A163[:, l, i0:i0 + dw],
                            ncs64[:, t * LC + l:t * LC + l + 1], None,
                            AluOpType.add)
                o0 = OUT_OFF[t]
                nc.sync.dma_start(out=out_d[:, o0:o0 + 4 * W],
                                  in_=oc[:, 0:4 * W])
                nc.gpsimd.dma_start(out=out_d[:, o0 + 4 * W:o0 + 8 * W],
                                    in_=oc[:, 4 * W:8 * W])

            for t in range(NST):
                i0 = t * P
                W = S - i0
                nw = min(NEARL, W)
                mw = min(DEEPL, W)
                oc = opool.tile([P, LC * W], BF16, name="oc%d" % t,
                                tag="oc%d" % t)
                oc3 = oc[:].rearrange("p (l j) -> p l j", j=W)
                ocs.append((oc, oc3))

                gA = g64[:, t * LC:t * LC + 4].rearrange(
                    "p l -> p l ()").broadcast_to([P, 4, mw])
                gBn = g64[:, t * LC + 4:t * LC + 8].rearrange(
                    "p l -> p l ()").broadcast_to([P, 4, nw])

                for l in range(4):
                    mm(t, l, SA, mw)
                # fused evacuation + min-G of the A half (mid cols too:
                # harmless, hnh << G there)
                nc.vector.tensor_tensor(oc3[:, 0:4, 0:mw],
                                        SA3[:, :, 0:mw], gA, AluOpType.min)
                for l in range(4, 8):
                    mm(t, l, SB, mw)
                nc.scalar.activation(oc3[:, 4:8, 0:mw], SB3[:, :, 0:mw],
                                     AF.Copy)
                nc.vector.tensor_tensor(oc3[:, 4:8, 0:nw], oc3[:, 4:8, 0:nw],
                                        gBn, AluOpType.min)
                if t == 0:
                    transposes(4, NST)
                if t > 0:
                    lagged(t - 1)
            lagged(NST - 1)

    nc.compile()
    return nc


def _bf16(a):
    u = np.ascontiguousarray(a, dtype=np.float32).view(np.uint32)
    r = ((u >> 16) & 1) + 0x7FFF
    return ((u + r) >> 16).astype(np.uint16)


def _unbf16(a):
    return (a.astype(np.uint32) << 16).view(np.float32)


def _host_inputs(x, W, b):
    """Per-core inputs. Core c: batch c//2, label half c%2."""
    x = np.asarray(x, dtype=np.float32)
    W = np.asarray(W, dtype=np.float32)
    b = np.asarray(b, dtype=np.float32)

    Wb = np.concatenate([W, b[None, :]], axis=0)          # (401, 65)
    wkp = np.zeros((4 * P, NT), np.float32)
    wkp[:H + 1] = Wb
    wk = _bf16(wkp.reshape(4, P, NT).transpose(1, 0, 2).reshape(P, 4 * NT))
    eye = np.eye(P, dtype=np.float32)
    sel2 = np.zeros((P, 96), np.float32)
    cols = np.concatenate([np.arange(8), 32 + np.arange(8), 64 + np.arange(8)])
    sel2[0, cols] = -1.0
    sel2[1 + np.arange(24), cols] = 1.0

    in_maps = []
    for c in range(8):
        bb, h = c // 2, c % 2
        xTb = np.concatenate([x[bb].T, np.ones((1, S), np.float32)], axis=0)
        xp = np.zeros((4 * P, S), np.float32)
        xp[:H + 1] = xTb
        xkc = _bf16(xp.reshape(4, P, S).transpose(1, 0, 2).reshape(P, 4 * S))
        sel = np.zeros((P, 32), np.float32)
        sel[:NT, 0] = 1.0
        for g in range(LC):
            lg = h * LC + g
            base = 1 + 4 * lg
            sel[base:base + 4, 1 + g] = 1.0          # I,B,L,U
            sel[[base + 1, base + 3], 9 + g] = 1.0   # B,U -> begin
            sel[[base + 2, base + 3], 17 + g] = 1.0  # L,U -> end
        in_maps.append({
            "xk": xkc, "Wk": wk, "selc": _bf16(sel),
            "sel2c": sel2.astype(np.float16), "eye": eye,
        })
    return in_maps


def kernel(x, mask, W, b, _collect=None):
    global _CACHED_NC
    if _CACHED_NC is None:
        _CACHED_NC = _build()
    nc = _CACHED_NC
    in_maps = _host_inputs(x, W, b)
    res = run_bass_kernel_spmd(nc, in_maps, list(range(8)))
    if _collect is not None:
        _collect.append(res)
    off = [0]
    for t in range(NST):
        off.append(off[-1] + LC * (S - P * t))
    outf = np.empty((B, S, S, NL), dtype=np.float32)
    for c in range(8):
        bb, h = c // 2, c % 2
        ofull = res.results[c]["out"]
        if ofull.dtype != np.uint16:
            ofull = ofull.view(np.uint16)
        for t in range(NST):
            i0 = t * P
            o = _unbf16(ofull[:, off[t]:off[t + 1]]).reshape(P, LC, S - i0)
            outf[bb, i0:i0 + P, i0:S, h * LC:(h + 1) * LC] = \
                o.transpose(0, 2, 1)
    # constant left/lower region (j < i) filled on host
    for i in range(1, S):
        outf[:, i, :i, :] = NEG
    return outf
